# revision 1
# baseline (speedup 1.0000x reference)
"""Block-causal attention TRN2 kernel (8-core SPMD, head-sharded).

Problem: y = (softmax(mask(Q K^T / sqrt(d))) V) W_out + b_out where
Q,K,V = x W_qkv + b_qkv, x [2, 2048, 1024], 16 heads of d=64, block-causal
mask with chunk 128.

Sharding: core c handles batch b = c//4 and head group g = c%4 (4 heads).
Each core computes its heads' QKV projection (W_qkv column slice), the
block-causal attention, and a partial out-projection against its W_out row
slice. The host sums the 4 partial outputs per batch and adds b_out.

On-device layout is "transposed": Q^T/K^T [d, L] tiles feed scores^T
matmuls (2 heads packed into the 128-partition contraction dim via PE row
groups), exp runs on the scalar engine with the 1/sqrt(d) scale folded in,
attn@V accumulates with an extra ones-column of V producing the softmax
denominators, and the normalized o^T directly feeds the out-projection as
the stationary operand. All matmul operands are float32r (~2e-4 rel err at
bf16 speed). The softmax reciprocal row is broadcast across partitions with
a K=1 PE matmul against a ones vector.

Everything runs in one pipelined loop over 512-row l/i-tiles: project tile
t, attend queries of tile t against key tiles 0..t, out-project tile t —
so PE, ACT, DVE and DMA work for different stages overlap.
"""

import sys

for _p in ("/opt/trn_rl_repo", "/root/.axon_site/_ro/trn_rl_repo"):
    if _p not in sys.path:
        sys.path.append(_p)

import numpy as np

import concourse.bass as bass
import concourse.mybir as mybir
import concourse.tile as tile
from concourse import bacc
from concourse.bass_utils import run_bass_kernel_spmd
from concourse.masks import make_identity

F32 = mybir.dt.float32
F32R = mybir.dt.float32r
EXP = mybir.ActivationFunctionType.Exp
ADD = mybir.AluOpType.add

B, L, D = 2, 2048, 1024
H, DH = 16, 64          # total heads, head dim
CHUNK = 128
HPC = 4                 # heads per core
S = HPC * DH            # 256 per-core qkv width per projection
N_CORES = 8
LT = 512                # l-tile (i-tile) size
NLT = L // LT           # 4
NKT = D // 128          # 8 k-tiles over D
NCT = 3 * S // 128      # 6 c-tiles (q pair0, q pair1, k p0, k p1, v p0, v p1)
NJT = L // CHUNK        # 16 j-tiles/chunks
SCALE = 1.0 / float(np.sqrt(DH))


def build_program(repeat=1):
    nc = bacc.Bacc("TRN2", target_bir_lowering=False, debug=False)
    x_d = nc.dram_tensor("x", [L, D], F32, kind="ExternalInput")
    w_d = nc.dram_tensor("w_qkv", [D, 3 * S], F32, kind="ExternalInput")
    bq_d = nc.dram_tensor("b_qkv", [3 * S], F32, kind="ExternalInput")
    wo_d = nc.dram_tensor("w_out", [S, D], F32, kind="ExternalInput")
    y_d = nc.dram_tensor("y", [L, D], F32, kind="ExternalOutput")

    with tile.TileContext(nc) as tc:
        lp = nc.allow_low_precision(reason="float32r matmul pipeline")
        lp.__enter__()
        with tc.tile_pool(name="const", bufs=1) as const, \
             tc.tile_pool(name="big", bufs=1) as big, \
             tc.tile_pool(name="stage", bufs=7) as stage, \
             tc.tile_pool(name="xtp", bufs=2) as xtp, \
             tc.tile_pool(name="expp", bufs=6) as expp, \
             tc.tile_pool(name="work", bufs=2) as work, \
             tc.tile_pool(name="small", bufs=1) as small, \
             tc.tile_pool(name="ps_tp", bufs=2, space="PSUM") as ps_tp, \
             tc.tile_pool(name="ps_pp", bufs=2, space="PSUM") as ps_pp, \
             tc.tile_pool(name="ps_s", bufs=2, space="PSUM") as ps_s, \
             tc.tile_pool(name="ps_o", bufs=2, space="PSUM") as ps_o:

            # ---- constants ----
            ident_f = const.tile([128, 128], F32)
            make_identity(nc, ident_f[:])
            identr = const.tile([128, 128], F32R)
            nc.vector.tensor_copy(identr[:], ident_f[:])
            ones_f = const.tile([128, 1], F32)
            nc.vector.memset(ones_f[:], 1.0)
            ones64 = const.tile([1, 64], F32R)
            o64f = const.tile([1, 64], F32)
            nc.vector.memset(o64f[:], 1.0)
            nc.vector.tensor_copy(ones64[:], o64f[:])
            # selector [1,128]: 0 for rows 0:64, 1 for rows 64:128
            sel128 = const.tile([1, 128], F32R)
            s128f = const.tile([1, 128], F32)
            nc.vector.memset(s128f[:, 0:64], 0.0)
            nc.vector.memset(s128f[:, 64:128], 1.0)
            nc.vector.tensor_copy(sel128[:], s128f[:])
            # b_qkv as per-c-tile per-partition bias columns [128, 6]
            bq_sb = const.tile([128, NCT], F32)
            bq_ap = bq_d.ap()
            nc.sync.dma_start(
                out=bq_sb[:],
                in_=bass.AP(tensor=bq_ap.tensor, offset=bq_ap.offset,
                            ap=[[1, 128], [128, NCT]]),
            )

            # ---- persistent weights/activations ----
            xs0 = []
            for sp in range(4):
                xst = stage.tile([128, D], F32R, tag="xs", name=f"xs0_{sp}")
                nc.sync.dma_start(
                    out=xst[:],
                    in_=x_d[sp * 128:(sp + 1) * 128, :].bitcast(F32R))
                xs0.append(xst)
            w_sb = big.tile([128, NKT, 3 * S], F32R)       # W_qkv k-tiles
            w_r = w_d.ap().rearrange("(kt p) c -> p kt c", p=128).bitcast(F32R)
            for ct in (0, 2, 4, 1, 3, 5):
                nc.sync.dma_start(
                    out=w_sb[:, :, ct * 128:(ct + 1) * 128],
                    in_=w_r[:, :, ct * 128:(ct + 1) * 128])
            wo_sb = big.tile([128, 2, D], F32R)            # W_out k-tiles (head pairs)
            for p in range(2):
                nc.sync.dma_start(out=wo_sb[:, p, :],
                                  in_=wo_d[p * 128:(p + 1) * 128, :].bitcast(F32R))
            qt_sb = big.tile([128, 2, L], F32R)            # Q^T pair-stacked
            kt_sb = big.tile([128, 2, L], F32R)            # K^T pair-stacked
            v_sb = big.tile([128, HPC, NJT, 128], F32R)    # V + ones col (odd heads padded)
            ot_sb = big.tile([128, 2, L], F32R)            # normalized o^T
            def init_v_const():
                for h in range(HPC):
                    col = 64 if h % 2 == 0 else 0
                    nc.vector.tensor_copy(
                        v_sb[:, h, :, col:col + 1],
                        bass.AP(tensor=ones_f.tensor, offset=ones_f.offset,
                                ap=ones_f.ap[:1] + [[0, NJT], [0, 1]]),
                    )
                    if h % 2 == 1:
                        nc.vector.memset(v_sb[:, h, :, 1:64].bitcast(F32), 0.0)

            def fetch_xs(rep, t):
                tiles = []
                for sp in range(4):
                    xst = stage.tile([128, D], F32R, tag="xs",
                                     name=f"rxs_{rep}_{t}_{sp}")
                    nc.sync.dma_start(
                        out=xst[:],
                        in_=x_d[t * LT + sp * 128: t * LT + (sp + 1) * 128,
                                :].bitcast(F32R))
                    tiles.append(xst)
                return tiles

            def emit_stage1(rep, t, xs):
                """x^T transposes for l-tile t; returns (xT, list of per-kt closures)."""
                l0 = t * LT
                xT = xtp.tile([128, NKT, LT], F32R, tag="xT", name=f"rxT_{rep}_{t}")
                units = []
                for kt in range(NKT):
                    def u(kt=kt, xT=xT, xs=xs, rep=rep, t=t):
                        tp = ps_tp.tile([128, LT], F32R, tag="tp",
                                        name=f"rtp_{rep}_{t}_{kt}")
                        for sp in range(4):
                            nc.tensor.transpose(
                                tp[:, sp * 128:(sp + 1) * 128],
                                xs[sp][:, kt * 128:(kt + 1) * 128], identr[:])
                        nc.vector.tensor_copy(xT[:, kt, :], tp[:])
                    units.append(u)
                return xT, units

            def emit_stage2(rep, t, xT):
                """QKV projection closures for l-tile t (6 c-tiles)."""
                l0 = t * LT
                units = []
                for ct in (0, 2, 4, 1, 3, 5):
                    def u(ct=ct, xT=xT, rep=rep, t=t, l0=l0):
                        pp = ps_pp.tile([128, LT], F32, tag="pp",
                                        name=f"rpp_{rep}_{t}_{ct}")
                        for kt in range(NKT):
                            nc.tensor.matmul(
                                pp[:], w_sb[:, kt, ct * 128:(ct + 1) * 128],
                                xT[:, kt, :],
                                start=(kt == 0), stop=(kt == NKT - 1))
                        if ct < 4:
                            dst = qt_sb if ct < 2 else kt_sb
                            nc.vector.tensor_scalar(
                                out=dst[:, ct % 2, l0:l0 + LT], in0=pp[:],
                                scalar1=bq_sb[:, ct:ct + 1], scalar2=None, op0=ADD)
                        else:
                            pv = ct - 4
                            vt_tmp = work.tile([128, LT], F32R, tag="vt_tmp",
                                               name=f"rvt_{rep}_{t}_{pv}")
                            nc.vector.tensor_scalar(
                                out=vt_tmp[:], in0=pp[:],
                                scalar1=bq_sb[:, ct:ct + 1], scalar2=None, op0=ADD)
                            tpv = ps_tp.tile([128, LT], F32R, tag="tp",
                                             name=f"rtpv_{rep}_{t}_{pv}")
                            for sp in range(4):
                                nc.tensor.transpose(
                                    tpv[:, sp * 128:(sp + 1) * 128],
                                    vt_tmp[:, sp * 128:(sp + 1) * 128], identr[:])
                            tpv_v = tpv[:].rearrange("j (sp h d) -> j sp h d",
                                                     sp=4, h=2)
                            for hh in range(2):
                                c0 = 0 if hh == 0 else 64
                                nc.vector.tensor_copy(
                                    v_sb[:, 2 * pv + hh, 4 * t:4 * (t + 1),
                                         c0:c0 + 64],
                                    tpv_v[:, :, hh, :])
                    units.append(u)
                return units

            def emit_outproj(rep, t):
                """Out-projection closures for i-tile t (8 units)."""
                units = []
                last = (t == NLT - 1)
                for st in range(4):
                    for mt in range(2):
                        def u(st=st, mt=mt, rep=rep, t=t, last=last):
                            i0 = t * LT + st * 128
                            yp = ps_pp.tile([128, 512], F32, tag="pp",
                                            name=f"ryp_{rep}_{t}_{st}_{mt}")
                            for p in range(2):
                                nc.tensor.matmul(
                                    yp[:], ot_sb[:, p, i0:i0 + 128],
                                    wo_sb[:, p, mt * 512:(mt + 1) * 512],
                                    start=(p == 0), stop=(p == 1))
                            y_sb = work.tile([128, 512], F32, tag="y_sb",
                                             name=f"rysb_{rep}_{t}_{st}_{mt}")
                            if last:
                                nc.scalar.copy(y_sb[:], yp[:])
                            else:
                                nc.vector.tensor_copy(y_sb[:], yp[:])
                            nc.sync.dma_start(
                                out=y_d[i0:i0 + 128, mt * 512:(mt + 1) * 512],
                                in_=y_sb[:])
                        units.append(u)
                return units

            def attn_scores_step(rep, t, p, jt, o_ps, s_pool, s_tag):
                l0 = t * LT
                njt = 4 * (t + 1)
                vis = max(0, jt - 4 * t) * 128
                s_pair = [s_pool.tile([128, LT], F32, tag=s_tag,
                                      name=f"rs_{p}_{rep}_{t}_{jt}_{hh}")
                          for hh in range(2)]
                for hh in range(2):
                    nc.tensor.matmul(
                        s_pair[hh][:, vis:LT],
                        kt_sb[hh * 64:(hh + 1) * 64, p,
                              jt * 128:(jt + 1) * 128],
                        qt_sb[hh * 64:(hh + 1) * 64, p,
                              l0 + vis:l0 + LT],
                        start=True, stop=True)
                for hh in range(2):
                    h = 2 * p + hh
                    e_t = expp.tile([128, LT], F32R, tag="e_t",
                                    name=f"re_{p}_{rep}_{t}_{jt}_{hh}")
                    nc.scalar.activation(
                        e_t[:, vis:LT], s_pair[hh][:, vis:LT],
                        EXP, scale=SCALE)
                    if hh == 0:
                        dst = o_ps[hh][0:65, vis:LT]
                        vw = v_sb[:, h, jt, 0:65]
                    else:
                        dst = o_ps[hh][0:128, vis:LT]
                        vw = v_sb[:, h, jt, 0:128]
                    nc.tensor.matmul(
                        dst, vw, e_t[:, vis:LT],
                        start=(jt == 0), stop=(jt == njt - 1))

            def attn_normalize(rep, t, p, o_ps):
                l0 = t * LT
                r2 = small.tile([1, 2, LT], F32R, tag="r2",
                                name=f"rr2_{p}_{rep}_{t}")
                nc.vector.reciprocal(r2[:, 0, :], o_ps[0][64:65, :])
                nc.vector.reciprocal(r2[:, 1, :], o_ps[1][0:1, :])
                rb = ps_s.tile([128, LT], F32, tag="s",
                               name=f"rrb_{p}_{rep}_{t}")
                nc.tensor.matmul(rb[:], sel128[:], r2[:, 1, :],
                                 start=True, stop=True)
                nc.tensor.matmul(rb[0:64, :], ones64[:], r2[:, 0, :],
                                 start=True, stop=True)
                rb_sb = work.tile([128, LT], F32, tag="rb_sb",
                                  name=f"rrbs_{p}_{rep}_{t}")
                nc.scalar.copy(rb_sb[:], rb[:])
                nc.vector.tensor_mul(
                    ot_sb[0:64, p, l0:l0 + LT],
                    o_ps[0][0:64, :], rb_sb[0:64, :])
                nc.vector.tensor_mul(
                    ot_sb[64:128, p, l0:l0 + LT],
                    o_ps[1][64:128, :], rb_sb[64:128, :])

            def emit_attention(rep, t, fillers):
                """Attention for i-tile t; drains one filler closure per j-tile."""
                for p in range(2):
                    o_ps = [ps_o.tile([128, LT], F32, tag="o_ps",
                                      name=f"ro_ps_{p}_{rep}_{t}_{hh}")
                            for hh in range(2)]
                    for jt in range(4 * (t + 1)):
                        attn_scores_step(rep, t, p, jt, o_ps, ps_s, "s")
                        if fillers:
                            fillers.pop(0)()
                    attn_normalize(rep, t, p, o_ps)

            def emit_attention_interleaved(rep, t, fillers):
                """Last-tile attention: both pairs' j-loops zipped; pair 1
                borrows the idle transpose/projection PSUM banks."""
                o_ps0 = [ps_o.tile([128, LT], F32, tag="o_ps",
                                   name=f"rio_{rep}_{t}_{hh}")
                         for hh in range(2)]
                o_ps1 = [ps_tp.tile([128, LT], F32, tag="tp",
                                    name=f"rio1_{rep}_{t}_{hh}")
                         for hh in range(2)]
                for jt in range(4 * (t + 1)):
                    attn_scores_step(rep, t, 0, jt, o_ps0, ps_s, "s")
                    if fillers:
                        fillers.pop(0)()
                    attn_scores_step(rep, t, 1, jt, o_ps1, ps_pp, "pp")
                attn_normalize(rep, t, 0, o_ps0)
                attn_normalize(rep, t, 1, o_ps1)

            xs_next = xs0
            for rep in range(repeat):
                pending = []
                a_emitted = False
                for t in range(NLT):
                    if not a_emitted:
                        xT, u1 = emit_stage1(rep, t, xs_next)
                        u2 = emit_stage2(rep, t, xT)
                        u1[0]()
                        if rep == 0:
                            init_v_const()
                        for u in u1[1:] + u2:
                            u()
                    # build next tile's projection units; they fill B(t) stalls
                    if t < NLT - 1:
                        xs_next = fetch_xs(rep, t + 1)
                        xTn, n1 = emit_stage1(rep, t + 1, xs_next)
                        n2 = emit_stage2(rep, t + 1, xTn)
                        pending.extend(n1 + n2)
                        a_emitted = True
                    else:
                        if rep < repeat - 1:
                            xs_next = fetch_xs(rep + 1, 0)
                        a_emitted = False
                    if t == NLT - 1:
                        emit_attention_interleaved(rep, t, pending)
                    else:
                        emit_attention(rep, t, pending)
                    for u in pending:
                        u()
                    pending.clear()
                    up = emit_outproj(rep, t)
                    if t == NLT - 1:
                        for u in up:
                            u()
                    else:
                        pending.extend(up)
        lp.__exit__(None, None, None)
    nc.compile()
    return nc


_NC_CACHE = {}


def _get_nc():
    if "nc" not in _NC_CACHE:
        _NC_CACHE["nc"] = build_program()
    return _NC_CACHE["nc"]


def make_in_maps(x, W_qkv, b_qkv, W_out):
    x = np.ascontiguousarray(np.asarray(x, dtype=np.float32))
    W_qkv = np.asarray(W_qkv, dtype=np.float32)
    b_qkv = np.asarray(b_qkv, dtype=np.float32)
    W_out = np.asarray(W_out, dtype=np.float32)
    in_maps = []
    for c in range(N_CORES):
        b, g = divmod(c, 4)
        cols = np.concatenate([np.arange(blk * D + g * S, blk * D + (g + 1) * S)
                               for blk in range(3)])
        in_maps.append({
            "x": np.ascontiguousarray(x[b]),
            "w_qkv": np.ascontiguousarray(W_qkv[:, cols]),
            "b_qkv": np.ascontiguousarray(b_qkv[cols]),
            "w_out": np.ascontiguousarray(W_out[g * S:(g + 1) * S, :]),
        })
    return in_maps


def kernel(x, W_qkv, b_qkv, W_out, b_out):
    nc = _get_nc()
    in_maps = make_in_maps(x, W_qkv, b_qkv, W_out)
    res = run_bass_kernel_spmd(nc, in_maps, list(range(N_CORES)))
    b_out = np.asarray(b_out, dtype=np.float32)
    out = np.zeros((B, L, D), dtype=np.float32)
    for c in range(N_CORES):
        out[c // 4] += res.results[c]["y"]
    out += b_out[None, None, :]
    return out



# revision 23
# speedup vs baseline: 462.5925x; 462.5925x over previous
"""Block-causal attention TRN2 kernel (8-core SPMD, head-sharded).

Problem: y = (softmax(mask(Q K^T / sqrt(d))) V) W_out + b_out where
Q,K,V = x W_qkv + b_qkv, x [2, 2048, 1024], 16 heads of d=64, block-causal
mask with chunk 128.

Sharding: core c handles batch b = c//4 and head group g = c%4 (4 heads).
Each core computes its heads' QKV projection (W_qkv column slice), the
block-causal attention, and a partial out-projection against its W_out row
slice. The host sums the 4 partial outputs per batch and adds b_out.

On-device layout is "transposed": Q^T/K^T [d, L] tiles feed scores^T
matmuls (2 heads packed into the 128-partition contraction dim via PE row
groups), exp runs on the scalar engine with the 1/sqrt(d) scale folded in,
attn@V accumulates with an extra ones-column of V producing the softmax
denominators, and the normalized o^T directly feeds the out-projection as
the stationary operand.

Dtypes: projection and out-projection matmuls run float32r (bf16-speed,
~2e-4 error); the attention working set (q^T/k^T/e/V) is bf16, which
streams narrow moving operands at 1 cycle/row where fp32r pays 4x, and
halves its SBUF/copy footprint. fp8 was tried and rejected: softmax output
is an average, so fp8's ~4% element noise survives 1:1 into the output
(~6e-2), over the 2e-2 budget.

Scheduling: the attention j-loop is software-pipelined one key-chunk ahead
— scores+exp for chunk jt+1 issue before attn@V of chunk jt, so the
scalar-engine exp latency hides behind PE matmuls instead of stalling the
attn@V accumulation (3 rotating PSUM score buffers make this legal).
Projection/transpose/out-projection work is chopped into small closures
drained one per chunk inside the attention loop, keeping PE busy during
the remaining exp/sem latency. Softmax normalization is split: the
denominator reciprocals issue right after the j-loop, but the PE broadcast
+ o^T scaling defer into the next pair's/tile's filler stream so nothing
ever head-of-line blocks the PE queue. With repeat>1 the loop is pipelined
across iterations (q/k/v double-buffered by repeat parity) so the next
iteration's projection fills the last tile's attention.
"""

import sys

for _p in ("/opt/trn_rl_repo", "/root/.axon_site/_ro/trn_rl_repo"):
    if _p not in sys.path:
        sys.path.append(_p)

import numpy as np

import concourse.bass as bass
import concourse.mybir as mybir
import concourse.tile as tile
from concourse import bacc
from concourse.bass_utils import run_bass_kernel_spmd
from concourse.masks import make_identity

F32 = mybir.dt.float32
F32R = mybir.dt.float32r
BF16 = mybir.dt.bfloat16
EXP = mybir.ActivationFunctionType.Exp
ADD = mybir.AluOpType.add

B, L, D = 2, 2048, 1024
H, DH = 16, 64          # total heads, head dim
CHUNK = 128
HPC = 4                 # heads per core
S = HPC * DH            # 256 per-core qkv width per projection
N_CORES = 8
LT = 512                # l-tile (i-tile) size
NLT = L // LT           # 4
NKT = D // 128          # 8 k-tiles over D
NCT = 3 * S // 128      # 6 c-tiles (q pair0, q pair1, k p0, k p1, v p0, v p1)
NJT = L // CHUNK        # 16 j-tiles/chunks
SCALE = 1.0 / float(np.sqrt(DH))


def build_program(repeat=1):
    nc = bacc.Bacc("TRN2", target_bir_lowering=False, debug=False)
    x_d = nc.dram_tensor("x", [L, D], F32, kind="ExternalInput")
    w_d = nc.dram_tensor("w_qkv", [D, 3 * S], F32, kind="ExternalInput")
    bq_d = nc.dram_tensor("b_qkv", [3 * S], F32, kind="ExternalInput")
    wo_d = nc.dram_tensor("w_out", [S, D], F32, kind="ExternalInput")
    y_d = nc.dram_tensor("y", [L, D], F32, kind="ExternalOutput")

    with tile.TileContext(nc) as tc:
        lp = nc.allow_low_precision(reason="bf16/f32r matmul pipeline")
        lp.__enter__()
        with tc.tile_pool(name="const", bufs=1) as const, \
             tc.tile_pool(name="big", bufs=1) as big, \
             tc.tile_pool(name="stage", bufs=7) as stage, \
             tc.tile_pool(name="xtp", bufs=2) as xtp, \
             tc.tile_pool(name="expp", bufs=4) as expp, \
             tc.tile_pool(name="work", bufs=2) as work, \
             tc.tile_pool(name="small", bufs=1) as small, \
             tc.tile_pool(name="ps_tp", bufs=1, space="PSUM") as ps_tp, \
             tc.tile_pool(name="ps_pp", bufs=2, space="PSUM") as ps_pp, \
             tc.tile_pool(name="ps_s", bufs=3, space="PSUM") as ps_s, \
             tc.tile_pool(name="ps_o", bufs=2, space="PSUM") as ps_o:

            # ---- constants ----
            ident_f = const.tile([128, 128], F32)
            make_identity(nc, ident_f[:])
            identr = const.tile([128, 128], F32R)
            nc.vector.tensor_copy(identr[:], ident_f[:])
            ones_f = const.tile([128, 1], F32)
            nc.vector.memset(ones_f[:], 1.0)
            ones64 = const.tile([1, 64], F32)
            nc.vector.memset(ones64[:], 1.0)
            ones64r = const.tile([1, 64], F32R)
            nc.vector.tensor_copy(ones64r[:], ones64[:])
            # selector [1,128]: 0 for rows 0:64, 1 for rows 64:128
            sel128 = const.tile([1, 128], F32)
            nc.vector.memset(sel128[:, 0:64], 0.0)
            nc.vector.memset(sel128[:, 64:128], 1.0)
            sel128r = const.tile([1, 128], F32R)
            nc.vector.tensor_copy(sel128r[:], sel128[:])
            # b_qkv as per-c-tile per-partition bias columns [128, 6]
            bq_sb = const.tile([128, NCT], F32)
            bq_ap = bq_d.ap()
            nc.sync.dma_start(
                out=bq_sb[:],
                in_=bass.AP(tensor=bq_ap.tensor, offset=bq_ap.offset,
                            ap=[[1, 128], [128, NCT]]),
            )

            # ---- persistent weights/activations ----
            xs0 = []
            for sp in range(4):
                xst = stage.tile([128, D], F32R, tag="xs", name=f"xs0_{sp}")
                nc.sync.dma_start(
                    out=xst[:],
                    in_=x_d[sp * 128:(sp + 1) * 128, :].bitcast(F32R))
                xs0.append(xst)
            w_sb = big.tile([128, NKT, 3 * S], F32R)       # W_qkv k-tiles
            w_r = w_d.ap().rearrange("(kt p) c -> p kt c", p=128).bitcast(F32R)
            for ct in (0, 2, 4, 1, 3, 5):
                nc.sync.dma_start(
                    out=w_sb[:, :, ct * 128:(ct + 1) * 128],
                    in_=w_r[:, :, ct * 128:(ct + 1) * 128])
            wo_sb = big.tile([128, 2, D], F32R)            # W_out k-tiles (head pairs)
            for p in range(2):
                nc.sync.dma_start(out=wo_sb[:, p, :],
                                  in_=wo_d[p * 128:(p + 1) * 128, :].bitcast(F32R))
            # double-buffered by repeat parity so iteration r+1's projection
            # can overlap iteration r's attention without hazards
            qt_sb = [big.tile([128, 2, L], BF16, name=f"qt{ph}")
                     for ph in range(2)]
            kt_sb = [big.tile([128, 2, L], BF16, name=f"kt{ph}")
                     for ph in range(2)]
            v_sb = [big.tile([128, HPC, NJT, 128], BF16, name=f"v{ph}")
                    for ph in range(2)]
            ot_sb = big.tile([128, 2, L], F32R)            # normalized o^T

            def init_v_const(ph):
                for h in range(HPC):
                    col = 64 if h % 2 == 0 else 0
                    nc.vector.tensor_copy(
                        v_sb[ph][:, h, :, col:col + 1],
                        bass.AP(tensor=ones_f.tensor, offset=ones_f.offset,
                                ap=ones_f.ap[:1] + [[0, NJT], [0, 1]]),
                    )
                    if h % 2 == 1:
                        nc.vector.memset(v_sb[ph][:, h, :, 1:64], 0.0)

            def fetch_xs(rep, t):
                tiles = []
                for sp in range(4):
                    xst = stage.tile([128, D], F32R, tag="xs",
                                     name=f"rxs_{rep}_{t}_{sp}")
                    nc.sync.dma_start(
                        out=xst[:],
                        in_=x_d[t * LT + sp * 128: t * LT + (sp + 1) * 128,
                                :].bitcast(F32R))
                    tiles.append(xst)
                return tiles

            def emit_stage1(rep, t, xs):
                """x^T transposes for l-tile t; one closure per k-tile."""
                xT = xtp.tile([128, NKT, LT], F32R, tag="xT", name=f"rxT_{rep}_{t}")
                units = []
                for kt in range(NKT):
                    def u(kt=kt, xT=xT, xs=xs, rep=rep, t=t):
                        tp = ps_tp.tile([128, LT], F32R, tag="tp",
                                        name=f"rtp_{rep}_{t}_{kt}")
                        for sp in range(4):
                            nc.tensor.transpose(
                                tp[:, sp * 128:(sp + 1) * 128],
                                xs[sp][:, kt * 128:(kt + 1) * 128], identr[:])
                        nc.vector.tensor_copy(xT[:, kt, :], tp[:])
                    units.append(u)
                return xT, units

            def emit_stage2(rep, t, xT, ph):
                """QKV projection closures for l-tile t.  Each c-tile is two
                closures (4 matmuls each) so fillers are fine-grained enough
                to slot one into every attention chunk."""
                l0 = t * LT
                units = []
                for ct in (0, 2, 4, 1, 3, 5):
                    holder = {}
                    def u_a(ct=ct, xT=xT, rep=rep, t=t, holder=holder):
                        pp = ps_pp.tile([128, LT], F32, tag="pp",
                                        name=f"rpp_{rep}_{t}_{ct}")
                        holder["pp"] = pp
                        for kt in range(NKT // 2):
                            nc.tensor.matmul(
                                pp[:], w_sb[:, kt, ct * 128:(ct + 1) * 128],
                                xT[:, kt, :],
                                start=(kt == 0), stop=False)
                    def u_b(ct=ct, xT=xT, rep=rep, t=t, l0=l0, ph=ph,
                            holder=holder):
                        pp = holder["pp"]
                        for kt in range(NKT // 2, NKT):
                            nc.tensor.matmul(
                                pp[:], w_sb[:, kt, ct * 128:(ct + 1) * 128],
                                xT[:, kt, :],
                                start=False, stop=(kt == NKT - 1))
                        if ct < 4:
                            dst = qt_sb[ph] if ct < 2 else kt_sb[ph]
                            nc.vector.tensor_scalar(
                                out=dst[:, ct % 2, l0:l0 + LT], in0=pp[:],
                                scalar1=bq_sb[:, ct:ct + 1], scalar2=None,
                                op0=ADD)
                        else:
                            pv = ct - 4
                            vt_tmp = work.tile([128, LT], F32R, tag="vt_tmp",
                                               name=f"rvt_{rep}_{t}_{pv}")
                            nc.vector.tensor_scalar(
                                out=vt_tmp[:], in0=pp[:],
                                scalar1=bq_sb[:, ct:ct + 1], scalar2=None,
                                op0=ADD)
                            tpv = ps_tp.tile([128, LT], F32R, tag="tp",
                                             name=f"rtpv_{rep}_{t}_{pv}")
                            for sp in range(4):
                                nc.tensor.transpose(
                                    tpv[:, sp * 128:(sp + 1) * 128],
                                    vt_tmp[:, sp * 128:(sp + 1) * 128],
                                    identr[:])
                            tpv_v = tpv[:].rearrange(
                                "j (sp h d) -> j sp h d", sp=4, h=2)
                            for hh in range(2):
                                c0 = 0 if hh == 0 else 64
                                nc.vector.tensor_copy(
                                    v_sb[ph][:, 2 * pv + hh, 4 * t:4 * (t + 1),
                                             c0:c0 + 64],
                                    tpv_v[:, :, hh, :])
                    units.append(u_a)
                    units.append(u_b)
                return units

            def emit_outproj(rep, t, last):
                """Out-projection closures for i-tile t (8 units)."""
                units = []
                for st in range(4):
                    for mt in range(2):
                        def u(st=st, mt=mt, rep=rep, t=t, last=last):
                            i0 = t * LT + st * 128
                            yp = ps_pp.tile([128, 512], F32, tag="pp",
                                            name=f"ryp_{rep}_{t}_{st}_{mt}")
                            for p in range(2):
                                nc.tensor.matmul(
                                    yp[:], ot_sb[:, p, i0:i0 + 128],
                                    wo_sb[:, p, mt * 512:(mt + 1) * 512],
                                    start=(p == 0), stop=(p == 1))
                            y_sb = work.tile([128, 512], F32, tag="y_sb",
                                             name=f"rysb_{rep}_{t}_{st}_{mt}")
                            if last:
                                nc.scalar.copy(y_sb[:], yp[:])
                            else:
                                nc.vector.tensor_copy(y_sb[:], yp[:])
                            nc.sync.dma_start(
                                out=y_d[i0:i0 + 128, mt * 512:(mt + 1) * 512],
                                in_=y_sb[:])
                        units.append(u)
                return units

            def sc_chunk(rep, t, p, jt, ph):
                """Scores + exp for one key chunk; returns the e tile."""
                l0 = t * LT
                vis = max(0, jt - 4 * t) * 128
                s_pair = [ps_s.tile([128, LT], F32, tag="s",
                                    name=f"rs_{p}_{rep}_{t}_{jt}_{hh}")
                          for hh in range(2)]
                for hh in range(2):
                    nc.tensor.matmul(
                        s_pair[hh][:, vis:LT],
                        kt_sb[ph][hh * 64:(hh + 1) * 64, p,
                                  jt * 128:(jt + 1) * 128],
                        qt_sb[ph][hh * 64:(hh + 1) * 64, p,
                                  l0 + vis:l0 + LT],
                        start=True, stop=True)
                e2 = expp.tile([128, 2, LT], BF16, tag="e_t",
                               name=f"re_{p}_{rep}_{t}_{jt}")
                for hh in range(2):
                    nc.scalar.activation(
                        e2[:, hh, vis:LT], s_pair[hh][:, vis:LT],
                        EXP, scale=SCALE)
                return e2

            def av_chunk(rep, t, p, jt, o_ps, ph, e2):
                njt = 4 * (t + 1)
                vis = max(0, jt - 4 * t) * 128
                for hh in range(2):
                    h = 2 * p + hh
                    if hh == 0:
                        dst = o_ps[hh][0:65, vis:LT]
                        vw = v_sb[ph][:, h, jt, 0:65]
                    else:
                        dst = o_ps[hh][0:128, vis:LT]
                        vw = v_sb[ph][:, h, jt, 0:128]
                    nc.tensor.matmul(
                        dst, vw, e2[:, hh, vis:LT],
                        start=(jt == 0), stop=(jt == njt - 1))

            def attn_normalize_split(rep, t, p, o_ps):
                """Issue the denominator reciprocals now (DVE, overlaps);
                return a closure with the PE broadcast + o^T scaling to be
                drained later in the filler stream."""
                l0 = t * LT
                r2r = small.tile([1, 2, LT], F32R, tag="r2r",
                                 name=f"rr2r_{p}_{rep}_{t}")
                nc.vector.reciprocal(r2r[:, 0, :], o_ps[0][64:65, :])
                nc.vector.reciprocal(r2r[:, 1, :], o_ps[1][0:1, :])

                def nb(rep=rep, t=t, p=p, o_ps=o_ps, r2r=r2r, l0=l0):
                    rb = ps_s.tile([128, LT], F32, tag="s",
                                   name=f"rrb_{p}_{rep}_{t}")
                    nc.tensor.matmul(rb[:], sel128r[:], r2r[:, 1, :],
                                     start=True, stop=True)
                    nc.tensor.matmul(rb[0:64, :], ones64r[:], r2r[:, 0, :],
                                     start=True, stop=True)
                    rb_sb = work.tile([128, LT], F32, tag="rb_sb",
                                      name=f"rrbs_{p}_{rep}_{t}")
                    nc.vector.tensor_copy(rb_sb[:], rb[:])
                    nc.vector.tensor_mul(
                        ot_sb[0:64, p, l0:l0 + LT],
                        o_ps[0][0:64, :], rb_sb[0:64, :])
                    nc.vector.tensor_mul(
                        ot_sb[64:128, p, l0:l0 + LT],
                        o_ps[1][64:128, :], rb_sb[64:128, :])
                return nb

            def emit_attention(rep, t, fillers, ph, deferred):
                """Attention for i-tile t, software-pipelined one chunk deep:
                scores+exp for chunk jt+1 issue before attn@V of chunk jt,
                with one filler closure drained in between."""
                njt = 4 * (t + 1)
                for p in range(2):
                    o_ps = [ps_o.tile([128, LT], F32, tag="o_ps",
                                      name=f"ro_ps_{p}_{rep}_{t}_{hh}")
                            for hh in range(2)]
                    e_prev = sc_chunk(rep, t, p, 0, ph)
                    for jt in range(njt):
                        e_next = (sc_chunk(rep, t, p, jt + 1, ph)
                                  if jt + 1 < njt else None)
                        if fillers:
                            fillers.pop(0)()
                        av_chunk(rep, t, p, jt, o_ps, ph, e_prev)
                        e_prev = e_next
                    nb = attn_normalize_split(rep, t, p, o_ps)
                    if p == 0:
                        fillers.insert(0, nb)
                    else:
                        deferred.append(nb)

            xs_next = xs0
            pending = []
            deferred = []
            a_emitted = False
            for rep in range(repeat):
                ph = rep % 2
                for t in range(NLT):
                    if not a_emitted:
                        xT, u1 = emit_stage1(rep, t, xs_next)
                        u2 = emit_stage2(rep, t, xT, ph)
                        u1[0]()
                        if rep == 0:
                            init_v_const(0)
                            if repeat > 1:
                                init_v_const(1)
                        for u in u1[1:] + u2:
                            u()
                    # build the next tile's (or next iteration's) projection
                    # units; they fill PE gaps inside this tile's attention
                    if t < NLT - 1:
                        xs_next = fetch_xs(rep, t + 1)
                        xTn, n1 = emit_stage1(rep, t + 1, xs_next)
                        n2 = emit_stage2(rep, t + 1, xTn, ph)
                        pending.extend(n1 + n2)
                        a_emitted = True
                    elif rep < repeat - 1:
                        xs_next = fetch_xs(rep + 1, 0)
                        xTn, n1 = emit_stage1(rep + 1, 0, xs_next)
                        n2 = emit_stage2(rep + 1, 0, xTn, 1 - ph)
                        pending.extend(n1 + n2)
                        a_emitted = True
                    else:
                        a_emitted = False
                    emit_attention(rep, t, pending, ph, deferred)
                    for u in pending:
                        u()
                    pending.clear()
                    final = (t == NLT - 1 and rep == repeat - 1)
                    up = emit_outproj(rep, t, final)
                    if final:
                        for u in deferred:
                            u()
                        deferred.clear()
                        for u in up:
                            u()
                    else:
                        pending.extend(deferred)
                        deferred.clear()
                        pending.extend(up)
        lp.__exit__(None, None, None)
    nc.compile()
    return nc


_NC_CACHE = {}


def _get_nc():
    if "nc" not in _NC_CACHE:
        _NC_CACHE["nc"] = build_program()
    return _NC_CACHE["nc"]


def make_in_maps(x, W_qkv, b_qkv, W_out):
    x = np.ascontiguousarray(np.asarray(x, dtype=np.float32))
    W_qkv = np.asarray(W_qkv, dtype=np.float32)
    b_qkv = np.asarray(b_qkv, dtype=np.float32)
    W_out = np.asarray(W_out, dtype=np.float32)
    in_maps = []
    for c in range(N_CORES):
        b, g = divmod(c, 4)
        cols = np.concatenate([np.arange(blk * D + g * S, blk * D + (g + 1) * S)
                               for blk in range(3)])
        in_maps.append({
            "x": np.ascontiguousarray(x[b]),
            "w_qkv": np.ascontiguousarray(W_qkv[:, cols]),
            "b_qkv": np.ascontiguousarray(b_qkv[cols]),
            "w_out": np.ascontiguousarray(W_out[g * S:(g + 1) * S, :]),
        })
    return in_maps


def kernel(x, W_qkv, b_qkv, W_out, b_out):
    nc = _get_nc()
    in_maps = make_in_maps(x, W_qkv, b_qkv, W_out)
    res = run_bass_kernel_spmd(nc, in_maps, list(range(N_CORES)))
    b_out = np.asarray(b_out, dtype=np.float32)
    out = np.zeros((B, L, D), dtype=np.float32)
    for c in range(N_CORES):
        out[c // 4] += res.results[c]["y"]
    out += b_out[None, None, :]
    return out


# revision 29
# speedup vs baseline: 475.2767x; 1.0274x over previous
"""Block-causal attention TRN2 kernel (8-core SPMD, head-sharded).

Problem: y = (softmax(mask(Q K^T / sqrt(d))) V) W_out + b_out where
Q,K,V = x W_qkv + b_qkv, x [2, 2048, 1024], 16 heads of d=64, block-causal
mask with chunk 128.

Sharding: core c handles batch b = c//4 and head group g = c%4 (4 heads).
Each core computes its heads' QKV projection (W_qkv column slice), the
block-causal attention, and a partial out-projection against its W_out row
slice. The host sums the 4 partial outputs per batch and adds b_out.

On-device layout is "transposed": Q^T/K^T [d, L] tiles feed scores^T
matmuls (2 heads packed into the 128-partition contraction dim via PE row
groups), exp runs on the scalar engine with the 1/sqrt(d) scale folded in,
attn@V accumulates with an extra ones-column of V producing the softmax
denominators, and the normalized o^T directly feeds the out-projection as
the stationary operand.

Dtypes: projection and out-projection matmuls run float32r (bf16-speed,
~2e-4 error); the attention working set (q^T/k^T/e/V) is bf16, which
streams narrow moving operands at 1 cycle/row where fp32r pays 4x, and
halves its SBUF/copy footprint. fp8 was tried and rejected: softmax output
is an average, so fp8's ~4% element noise survives 1:1 into the output
(~6e-2), over the 2e-2 budget.

Scheduling: the attention j-loop is software-pipelined one key-chunk ahead
— scores+exp for chunk jt+1 issue before attn@V of chunk jt, so the
scalar-engine exp latency hides behind PE matmuls instead of stalling the
attn@V accumulation (3 rotating PSUM score buffers make this legal).
Projection/transpose/out-projection work is chopped into small closures
drained one per chunk inside the attention loop, keeping PE busy during
the remaining exp/sem latency. Softmax normalization is split: the
denominator reciprocals issue right after the j-loop, but the PE broadcast
+ o^T scaling defer into the next pair's/tile's filler stream so nothing
ever head-of-line blocks the PE queue. With repeat>1 the loop is pipelined
across iterations (q/k/v double-buffered by repeat parity) so the next
iteration's projection fills the last tile's attention.
"""

import sys

for _p in ("/opt/trn_rl_repo", "/root/.axon_site/_ro/trn_rl_repo"):
    if _p not in sys.path:
        sys.path.append(_p)

import numpy as np

import concourse.bass as bass
import concourse.mybir as mybir
import concourse.tile as tile
from concourse import bacc
from concourse.bass_utils import run_bass_kernel_spmd
from concourse.masks import make_identity

F32 = mybir.dt.float32
F32R = mybir.dt.float32r
BF16 = mybir.dt.bfloat16
EXP = mybir.ActivationFunctionType.Exp
ADD = mybir.AluOpType.add

B, L, D = 2, 2048, 1024
H, DH = 16, 64          # total heads, head dim
CHUNK = 128
HPC = 4                 # heads per core
S = HPC * DH            # 256 per-core qkv width per projection
N_CORES = 8
LT = 512                # l-tile (i-tile) size
NLT = L // LT           # 4
NKT = D // 128          # 8 k-tiles over D
NCT = 3 * S // 128      # 6 c-tiles (q pair0, q pair1, k p0, k p1, v p0, v p1)
NJT = L // CHUNK        # 16 j-tiles/chunks
SCALE = 1.0 / float(np.sqrt(DH))


def build_program(repeat=1):
    nc = bacc.Bacc("TRN2", target_bir_lowering=False, debug=False)
    x_d = nc.dram_tensor("x", [L, D], F32, kind="ExternalInput")
    w_d = nc.dram_tensor("w_qkv", [D, 3 * S], F32, kind="ExternalInput")
    bq_d = nc.dram_tensor("b_qkv", [3 * S], F32, kind="ExternalInput")
    wo_d = nc.dram_tensor("w_out", [S, D], F32, kind="ExternalInput")
    y_d = nc.dram_tensor("y", [L, D], F32, kind="ExternalOutput")

    with tile.TileContext(nc) as tc:
        lp = nc.allow_low_precision(reason="bf16/f32r matmul pipeline")
        lp.__enter__()
        with tc.tile_pool(name="const", bufs=1) as const, \
             tc.tile_pool(name="big", bufs=1) as big, \
             tc.tile_pool(name="stage", bufs=7) as stage, \
             tc.tile_pool(name="xtp", bufs=2) as xtp, \
             tc.tile_pool(name="expp", bufs=4) as expp, \
             tc.tile_pool(name="work", bufs=2) as work, \
             tc.tile_pool(name="small", bufs=1) as small, \
             tc.tile_pool(name="ps_pp", bufs=2, space="PSUM") as ps_pp, \
             tc.tile_pool(name="ps_s", bufs=4, space="PSUM") as ps_s, \
             tc.tile_pool(name="ps_o", bufs=2, space="PSUM") as ps_o:
            # transpose scratch shares the scores ring (same 2KB bank shape):
            # one pool, one tag, 4 rotating banks for scores + transposes + rb
            ps_tp = ps_s

            # ---- constants ----
            ident_f = const.tile([128, 128], F32)
            make_identity(nc, ident_f[:])
            identr = const.tile([128, 128], F32R)
            nc.vector.tensor_copy(identr[:], ident_f[:])
            ones_f = const.tile([128, 1], F32)
            nc.vector.memset(ones_f[:], 1.0)
            ones64 = const.tile([1, 64], F32)
            nc.vector.memset(ones64[:], 1.0)
            ones64r = const.tile([1, 64], F32R)
            nc.vector.tensor_copy(ones64r[:], ones64[:])
            # selector [1,128]: 0 for rows 0:64, 1 for rows 64:128
            sel128 = const.tile([1, 128], F32)
            nc.vector.memset(sel128[:, 0:64], 0.0)
            nc.vector.memset(sel128[:, 64:128], 1.0)
            sel128r = const.tile([1, 128], F32R)
            nc.vector.tensor_copy(sel128r[:], sel128[:])
            # b_qkv as per-c-tile per-partition bias columns [128, 6]
            bq_sb = const.tile([128, NCT], F32)
            bq_ap = bq_d.ap()
            nc.sync.dma_start(
                out=bq_sb[:],
                in_=bass.AP(tensor=bq_ap.tensor, offset=bq_ap.offset,
                            ap=[[1, 128], [128, NCT]]),
            )

            # ---- persistent weights/activations ----
            xs0 = []
            for sp in range(4):
                xst = stage.tile([128, D], F32R, tag="xs", name=f"xs0_{sp}")
                nc.sync.dma_start(
                    out=xst[:],
                    in_=x_d[sp * 128:(sp + 1) * 128, :].bitcast(F32R))
                xs0.append(xst)
            w_sb = big.tile([128, NKT, 3 * S], F32R)       # W_qkv k-tiles
            w_r = w_d.ap().rearrange("(kt p) c -> p kt c", p=128).bitcast(F32R)
            for ct in (0, 2, 4, 1, 3, 5):
                nc.sync.dma_start(
                    out=w_sb[:, :, ct * 128:(ct + 1) * 128],
                    in_=w_r[:, :, ct * 128:(ct + 1) * 128])
            wo_sb = big.tile([128, 2, D], F32R)            # W_out k-tiles (head pairs)
            for p in range(2):
                nc.sync.dma_start(out=wo_sb[:, p, :],
                                  in_=wo_d[p * 128:(p + 1) * 128, :].bitcast(F32R))
            # double-buffered by repeat parity so iteration r+1's projection
            # can overlap iteration r's attention without hazards
            qt_sb = [big.tile([128, 2, L], BF16, name=f"qt{ph}")
                     for ph in range(2)]
            kt_sb = [big.tile([128, 2, L], BF16, name=f"kt{ph}")
                     for ph in range(2)]
            v_sb = [big.tile([128, HPC, NJT, 128], BF16, name=f"v{ph}")
                    for ph in range(2)]
            ot_sb = big.tile([128, 2, L], F32R)            # normalized o^T

            def init_v_const(ph):
                for h in range(HPC):
                    col = 64 if h % 2 == 0 else 0
                    nc.vector.tensor_copy(
                        v_sb[ph][:, h, :, col:col + 1],
                        bass.AP(tensor=ones_f.tensor, offset=ones_f.offset,
                                ap=ones_f.ap[:1] + [[0, NJT], [0, 1]]),
                    )
                    if h % 2 == 1:
                        nc.vector.memset(v_sb[ph][:, h, :, 1:64], 0.0)

            def fetch_xs(rep, t):
                tiles = []
                for sp in range(4):
                    xst = stage.tile([128, D], F32R, tag="xs",
                                     name=f"rxs_{rep}_{t}_{sp}")
                    nc.sync.dma_start(
                        out=xst[:],
                        in_=x_d[t * LT + sp * 128: t * LT + (sp + 1) * 128,
                                :].bitcast(F32R))
                    tiles.append(xst)
                return tiles

            def emit_stage1(rep, t, xs):
                """x^T transposes for l-tile t; one closure per k-tile."""
                xT = xtp.tile([128, NKT, LT], F32R, tag="xT", name=f"rxT_{rep}_{t}")
                units = []
                for kt in range(NKT):
                    def u(kt=kt, xT=xT, xs=xs, rep=rep, t=t):
                        tp = ps_tp.tile([128, LT], F32R, tag="s",
                                        name=f"rtp_{rep}_{t}_{kt}")
                        for sp in range(4):
                            nc.tensor.transpose(
                                tp[:, sp * 128:(sp + 1) * 128],
                                xs[sp][:, kt * 128:(kt + 1) * 128], identr[:])
                        nc.vector.tensor_copy(xT[:, kt, :], tp[:])
                    units.append(u)
                return xT, units

            def emit_stage2(rep, t, xT, ph):
                """QKV projection closures for l-tile t.  Each c-tile is two
                closures (4 matmuls each) so fillers are fine-grained enough
                to slot one into every attention chunk."""
                l0 = t * LT
                units = []
                for ct in (0, 2, 4, 1, 3, 5):
                    holder = {}
                    def u_a(ct=ct, xT=xT, rep=rep, t=t, holder=holder):
                        pp = ps_pp.tile([128, LT], F32, tag="pp",
                                        name=f"rpp_{rep}_{t}_{ct}")
                        holder["pp"] = pp
                        for kt in range(NKT // 2):
                            nc.tensor.matmul(
                                pp[:], w_sb[:, kt, ct * 128:(ct + 1) * 128],
                                xT[:, kt, :],
                                start=(kt == 0), stop=False)
                    def u_b(ct=ct, xT=xT, rep=rep, t=t, l0=l0, ph=ph,
                            holder=holder):
                        pp = holder["pp"]
                        for kt in range(NKT // 2, NKT):
                            nc.tensor.matmul(
                                pp[:], w_sb[:, kt, ct * 128:(ct + 1) * 128],
                                xT[:, kt, :],
                                start=False, stop=(kt == NKT - 1))
                        if ct < 4:
                            dst = qt_sb[ph] if ct < 2 else kt_sb[ph]
                            nc.vector.tensor_scalar(
                                out=dst[:, ct % 2, l0:l0 + LT], in0=pp[:],
                                scalar1=bq_sb[:, ct:ct + 1], scalar2=None,
                                op0=ADD)
                        else:
                            pv = ct - 4
                            vt_tmp = work.tile([128, LT], F32R, tag="vt_tmp",
                                               name=f"rvt_{rep}_{t}_{pv}")
                            nc.vector.tensor_scalar(
                                out=vt_tmp[:], in0=pp[:],
                                scalar1=bq_sb[:, ct:ct + 1], scalar2=None,
                                op0=ADD)
                            tpv = ps_tp.tile([128, LT], F32R, tag="s",
                                             name=f"rtpv_{rep}_{t}_{pv}")
                            for sp in range(4):
                                nc.tensor.transpose(
                                    tpv[:, sp * 128:(sp + 1) * 128],
                                    vt_tmp[:, sp * 128:(sp + 1) * 128],
                                    identr[:])
                            tpv_v = tpv[:].rearrange(
                                "j (sp h d) -> j sp h d", sp=4, h=2)
                            for hh in range(2):
                                c0 = 0 if hh == 0 else 64
                                nc.vector.tensor_copy(
                                    v_sb[ph][:, 2 * pv + hh, 4 * t:4 * (t + 1),
                                             c0:c0 + 64],
                                    tpv_v[:, :, hh, :])
                    units.append(u_a)
                    units.append(u_b)
                return units

            def emit_outproj(rep, t, last):
                """Out-projection closures for i-tile t (8 units)."""
                units = []
                for st in range(4):
                    for mt in range(2):
                        def u(st=st, mt=mt, rep=rep, t=t, last=last):
                            i0 = t * LT + st * 128
                            yp = ps_pp.tile([128, 512], F32, tag="pp",
                                            name=f"ryp_{rep}_{t}_{st}_{mt}")
                            for p in range(2):
                                nc.tensor.matmul(
                                    yp[:], ot_sb[:, p, i0:i0 + 128],
                                    wo_sb[:, p, mt * 512:(mt + 1) * 512],
                                    start=(p == 0), stop=(p == 1))
                            y_sb = work.tile([128, 512], F32, tag="y_sb",
                                             name=f"rysb_{rep}_{t}_{st}_{mt}")
                            if last:
                                nc.scalar.copy(y_sb[:], yp[:])
                            else:
                                nc.vector.tensor_copy(y_sb[:], yp[:])
                            nc.sync.dma_start(
                                out=y_d[i0:i0 + 128, mt * 512:(mt + 1) * 512],
                                in_=y_sb[:])
                        units.append(u)
                return units

            def sc_chunk(rep, t, p, jt, ph):
                """Scores + exp for one key chunk; returns the e tile."""
                l0 = t * LT
                vis = max(0, jt - 4 * t) * 128
                s_pair = [ps_s.tile([128, LT], F32, tag="s",
                                    name=f"rs_{p}_{rep}_{t}_{jt}_{hh}")
                          for hh in range(2)]
                for hh in range(2):
                    nc.tensor.matmul(
                        s_pair[hh][:, vis:LT],
                        kt_sb[ph][hh * 64:(hh + 1) * 64, p,
                                  jt * 128:(jt + 1) * 128],
                        qt_sb[ph][hh * 64:(hh + 1) * 64, p,
                                  l0 + vis:l0 + LT],
                        start=True, stop=True)
                e2 = expp.tile([128, 2, LT], BF16, tag="e_t",
                               name=f"re_{p}_{rep}_{t}_{jt}")
                for hh in range(2):
                    nc.scalar.activation(
                        e2[:, hh, vis:LT], s_pair[hh][:, vis:LT],
                        EXP, scale=SCALE)
                return e2

            def av_chunk(rep, t, p, jt, o_ps, ph, e2):
                njt = 4 * (t + 1)
                vis = max(0, jt - 4 * t) * 128
                for hh in range(2):
                    h = 2 * p + hh
                    if hh == 0:
                        dst = o_ps[hh][0:65, vis:LT]
                        vw = v_sb[ph][:, h, jt, 0:65]
                    else:
                        dst = o_ps[hh][0:128, vis:LT]
                        vw = v_sb[ph][:, h, jt, 0:128]
                    nc.tensor.matmul(
                        dst, vw, e2[:, hh, vis:LT],
                        start=(jt == 0), stop=(jt == njt - 1))

            def attn_normalize_split(rep, t, p, o_ps):
                """Issue the denominator reciprocals now (DVE, overlaps);
                return a closure with the PE broadcast + o^T scaling to be
                drained later in the filler stream."""
                l0 = t * LT
                r2r = small.tile([1, 2, LT], F32R, tag="r2r",
                                 name=f"rr2r_{p}_{rep}_{t}")
                nc.vector.reciprocal(r2r[:, 0, :], o_ps[0][64:65, :])
                nc.vector.reciprocal(r2r[:, 1, :], o_ps[1][0:1, :])

                def nb(rep=rep, t=t, p=p, o_ps=o_ps, r2r=r2r, l0=l0):
                    rb = ps_s.tile([128, LT], F32, tag="s",
                                   name=f"rrb_{p}_{rep}_{t}")
                    nc.tensor.matmul(rb[:], sel128r[:], r2r[:, 1, :],
                                     start=True, stop=True)
                    nc.tensor.matmul(rb[0:64, :], ones64r[:], r2r[:, 0, :],
                                     start=True, stop=True)
                    rb_sb = work.tile([128, LT], F32, tag="rb_sb",
                                      name=f"rrbs_{p}_{rep}_{t}")
                    nc.vector.tensor_copy(rb_sb[:], rb[:])
                    nc.vector.tensor_mul(
                        ot_sb[0:64, p, l0:l0 + LT],
                        o_ps[0][0:64, :], rb_sb[0:64, :])
                    nc.vector.tensor_mul(
                        ot_sb[64:128, p, l0:l0 + LT],
                        o_ps[1][64:128, :], rb_sb[64:128, :])
                return nb

            def emit_attention(rep, t, fillers, ph, deferred):
                """Attention for i-tile t, software-pipelined two chunks deep:
                scores+exp for chunk jt+2 issue before attn@V of chunk jt
                (4 rotating PSUM score buffers make this legal), so the exp
                and its semaphore hops complete ~2 chunks of PE work early."""
                njt = 4 * (t + 1)
                for p in range(2):
                    o_ps = [ps_o.tile([128, LT], F32, tag="o_ps",
                                      name=f"ro_ps_{p}_{rep}_{t}_{hh}")
                            for hh in range(2)]
                    e_q = [sc_chunk(rep, t, p, 0, ph)]
                    for jt in range(njt):
                        if jt + 1 < njt:
                            e_q.append(sc_chunk(rep, t, p, jt + 1, ph))
                        if fillers:
                            fillers.pop(0)()
                        av_chunk(rep, t, p, jt, o_ps, ph, e_q.pop(0))
                    nb = attn_normalize_split(rep, t, p, o_ps)
                    if p == 0:
                        fillers.insert(0, nb)
                    else:
                        deferred.append(nb)

            xs_next = xs0
            pending = []
            deferred = []
            a_emitted = False
            for rep in range(repeat):
                ph = rep % 2
                for t in range(NLT):
                    if not a_emitted:
                        xT, u1 = emit_stage1(rep, t, xs_next)
                        u2 = emit_stage2(rep, t, xT, ph)
                        u1[0]()
                        if rep == 0:
                            init_v_const(0)
                            if repeat > 1:
                                init_v_const(1)
                        for u in u1[1:] + u2:
                            u()
                    # build the next tile's (or next iteration's) projection
                    # units; they fill PE gaps inside this tile's attention.
                    # Splice two transpose units ahead of the deferred
                    # out-projection so the latter isn't drained right on the
                    # heels of the normalize closure it depends on.
                    if t < NLT - 1:
                        xs_next = fetch_xs(rep, t + 1)
                        xTn, n1 = emit_stage1(rep, t + 1, xs_next)
                        n2 = emit_stage2(rep, t + 1, xTn, ph)
                        pending[1:1] = n1[:2]
                        pending.extend(n1[2:] + n2)
                        a_emitted = True
                    elif rep < repeat - 1:
                        xs_next = fetch_xs(rep + 1, 0)
                        xTn, n1 = emit_stage1(rep + 1, 0, xs_next)
                        n2 = emit_stage2(rep + 1, 0, xTn, 1 - ph)
                        pending[1:1] = n1[:2]
                        pending.extend(n1[2:] + n2)
                        a_emitted = True
                    else:
                        a_emitted = False
                    emit_attention(rep, t, pending, ph, deferred)
                    for u in pending:
                        u()
                    pending.clear()
                    final = (t == NLT - 1 and rep == repeat - 1)
                    up = emit_outproj(rep, t, final)
                    if final:
                        for u in deferred:
                            u()
                        deferred.clear()
                        for u in up:
                            u()
                    else:
                        pending.extend(deferred)
                        deferred.clear()
                        pending.extend(up)
        lp.__exit__(None, None, None)
    nc.compile()
    return nc


_NC_CACHE = {}


def _get_nc():
    if "nc" not in _NC_CACHE:
        _NC_CACHE["nc"] = build_program()
    return _NC_CACHE["nc"]


def make_in_maps(x, W_qkv, b_qkv, W_out):
    x = np.ascontiguousarray(np.asarray(x, dtype=np.float32))
    W_qkv = np.asarray(W_qkv, dtype=np.float32)
    b_qkv = np.asarray(b_qkv, dtype=np.float32)
    W_out = np.asarray(W_out, dtype=np.float32)
    in_maps = []
    for c in range(N_CORES):
        b, g = divmod(c, 4)
        cols = np.concatenate([np.arange(blk * D + g * S, blk * D + (g + 1) * S)
                               for blk in range(3)])
        in_maps.append({
            "x": np.ascontiguousarray(x[b]),
            "w_qkv": np.ascontiguousarray(W_qkv[:, cols]),
            "b_qkv": np.ascontiguousarray(b_qkv[cols]),
            "w_out": np.ascontiguousarray(W_out[g * S:(g + 1) * S, :]),
        })
    return in_maps


def kernel(x, W_qkv, b_qkv, W_out, b_out):
    nc = _get_nc()
    in_maps = make_in_maps(x, W_qkv, b_qkv, W_out)
    res = run_bass_kernel_spmd(nc, in_maps, list(range(N_CORES)))
    b_out = np.asarray(b_out, dtype=np.float32)
    out = np.zeros((B, L, D), dtype=np.float32)
    for c in range(N_CORES):
        out[c // 4] += res.results[c]["y"]
    out += b_out[None, None, :]
    return out


# revision 34
# speedup vs baseline: 509.1420x; 1.0713x over previous
"""Block-causal attention TRN2 kernel (8-core SPMD, head-sharded).

Problem: y = (softmax(mask(Q K^T / sqrt(d))) V) W_out + b_out where
Q,K,V = x W_qkv + b_qkv, x [2, 2048, 1024], 16 heads of d=64, block-causal
mask with chunk 128.

Sharding: core c handles batch b = c//4 and head group g = c%4 (4 heads).
Each core computes its heads' QKV projection (W_qkv column slice), the
block-causal attention, and a partial out-projection against its W_out row
slice. The host sums the 4 partial outputs per batch and adds b_out.

On-device layout is "transposed": Q^T/K^T [d, L] tiles feed scores^T
matmuls (2 heads packed into the 128-partition contraction dim via PE row
groups), exp runs on the scalar engine with the 1/sqrt(d) scale folded in,
attn@V accumulates with an extra ones-column of V producing the softmax
denominators, and the normalized o^T directly feeds the out-projection as
the stationary operand.

Dtypes: projection and out-projection matmuls run float32r (bf16-speed,
~2e-4 error); the attention working set (q^T/k^T/e/V) is bf16, which
streams narrow moving operands at 1 cycle/row where fp32r pays 4x, and
halves its SBUF/copy footprint. fp8 was tried and rejected: softmax output
is an average, so fp8's ~4% element noise survives 1:1 into the output
(~6e-2), over the 2e-2 budget.

Scheduling: the attention j-loop is software-pipelined one key-chunk ahead
— scores+exp for chunk jt+1 issue before attn@V of chunk jt, so the
scalar-engine exp latency hides behind PE matmuls instead of stalling the
attn@V accumulation (3 rotating PSUM score buffers make this legal).
Projection/transpose/out-projection work is chopped into small closures
drained one per chunk inside the attention loop, keeping PE busy during
the remaining exp/sem latency. Softmax normalization is split: the
denominator reciprocals issue right after the j-loop, but the PE broadcast
+ o^T scaling defer into the next pair's/tile's filler stream so nothing
ever head-of-line blocks the PE queue. With repeat>1 the loop is pipelined
across iterations (q/k/v double-buffered by repeat parity) so the next
iteration's projection fills the last tile's attention.
"""

import sys

for _p in ("/opt/trn_rl_repo", "/root/.axon_site/_ro/trn_rl_repo"):
    if _p not in sys.path:
        sys.path.append(_p)

import numpy as np

import concourse.bass as bass
import concourse.mybir as mybir
import concourse.tile as tile
from concourse import bacc
from concourse.bass_utils import run_bass_kernel_spmd
from concourse.masks import make_identity

F32 = mybir.dt.float32
F32R = mybir.dt.float32r
BF16 = mybir.dt.bfloat16
EXP = mybir.ActivationFunctionType.Exp
ADD = mybir.AluOpType.add

B, L, D = 2, 2048, 1024
H, DH = 16, 64          # total heads, head dim
CHUNK = 128
HPC = 4                 # heads per core
S = HPC * DH            # 256 per-core qkv width per projection
N_CORES = 8
LT = 512                # l-tile (i-tile) size
NLT = L // LT           # 4
NKT = D // 128          # 8 k-tiles over D
NCT = 3 * S // 128      # 6 c-tiles (q pair0, q pair1, k p0, k p1, v p0, v p1)
NJT = L // CHUNK        # 16 j-tiles/chunks
SCALE = 1.0 / float(np.sqrt(DH))


def build_program(repeat=1):
    nc = bacc.Bacc("TRN2", target_bir_lowering=False, debug=False)
    x_d = nc.dram_tensor("x", [L, D], F32, kind="ExternalInput")
    w_d = nc.dram_tensor("w_qkv", [D, 3 * S], F32, kind="ExternalInput")
    bq_d = nc.dram_tensor("b_qkv", [3 * S], F32, kind="ExternalInput")
    wo_d = nc.dram_tensor("w_out", [S, D], F32, kind="ExternalInput")
    y_d = nc.dram_tensor("y", [L, D], F32, kind="ExternalOutput")

    with tile.TileContext(nc) as tc:
        lp = nc.allow_low_precision(reason="bf16/f32r matmul pipeline")
        lp.__enter__()
        with tc.tile_pool(name="const", bufs=1) as const, \
             tc.tile_pool(name="big", bufs=1) as big, \
             tc.tile_pool(name="stage", bufs=7) as stage, \
             tc.tile_pool(name="xtp", bufs=2) as xtp, \
             tc.tile_pool(name="expp", bufs=4) as expp, \
             tc.tile_pool(name="work", bufs=2) as work, \
             tc.tile_pool(name="small", bufs=1) as small, \
             tc.tile_pool(name="ps_pp", bufs=2, space="PSUM") as ps_pp, \
             tc.tile_pool(name="ps_s", bufs=2, space="PSUM") as ps_s, \
             tc.tile_pool(name="ps_o", bufs=2, space="PSUM") as ps_o:
            # transpose scratch shares the scores ring: one pool, one tag,
            # 2 rotating 2-bank slots for scores + transposes + rb
            ps_tp = ps_s

            # ---- constants ----
            ident_f = const.tile([128, 128], F32)
            make_identity(nc, ident_f[:])
            identr = const.tile([128, 128], F32R)
            nc.vector.tensor_copy(identr[:], ident_f[:])
            ones_f = const.tile([128, 1], F32)
            nc.vector.memset(ones_f[:], 1.0)
            ones64 = const.tile([1, 64], F32)
            nc.vector.memset(ones64[:], 1.0)
            ones64r = const.tile([1, 64], F32R)
            nc.vector.tensor_copy(ones64r[:], ones64[:])
            # selector [1,128]: 0 for rows 0:64, 1 for rows 64:128
            sel128 = const.tile([1, 128], F32)
            nc.vector.memset(sel128[:, 0:64], 0.0)
            nc.vector.memset(sel128[:, 64:128], 1.0)
            sel128r = const.tile([1, 128], F32R)
            nc.vector.tensor_copy(sel128r[:], sel128[:])
            # b_qkv as per-c-tile per-partition bias columns [128, 6]
            bq_sb = const.tile([128, NCT], F32)
            bq_ap = bq_d.ap()
            nc.sync.dma_start(
                out=bq_sb[:],
                in_=bass.AP(tensor=bq_ap.tensor, offset=bq_ap.offset,
                            ap=[[1, 128], [128, NCT]]),
            )

            # ---- persistent weights/activations ----
            xs0 = []
            for sp in range(4):
                xst = stage.tile([128, D], F32R, tag="xs", name=f"xs0_{sp}")
                nc.sync.dma_start(
                    out=xst[:],
                    in_=x_d[sp * 128:(sp + 1) * 128, :].bitcast(F32R))
                xs0.append(xst)
            w_sb = big.tile([128, NKT, 3 * S], F32R)       # W_qkv k-tiles
            w_r = w_d.ap().rearrange("(kt p) c -> p kt c", p=128).bitcast(F32R)
            for ct in (0, 2, 4, 1, 3, 5):
                nc.sync.dma_start(
                    out=w_sb[:, :, ct * 128:(ct + 1) * 128],
                    in_=w_r[:, :, ct * 128:(ct + 1) * 128])
            wo_sb = big.tile([128, 2, D], F32R)            # W_out k-tiles (head pairs)
            for p in range(2):
                nc.sync.dma_start(out=wo_sb[:, p, :],
                                  in_=wo_d[p * 128:(p + 1) * 128, :].bitcast(F32R))
            # double-buffered by repeat parity so iteration r+1's projection
            # can overlap iteration r's attention without hazards
            qt_sb = [big.tile([128, 2, L], BF16, name=f"qt{ph}")
                     for ph in range(2)]
            kt_sb = [big.tile([128, 2, L], BF16, name=f"kt{ph}")
                     for ph in range(2)]
            v_sb = [big.tile([128, HPC, NJT, 128], BF16, name=f"v{ph}")
                    for ph in range(2)]
            ot_sb = big.tile([128, 2, L], F32R)            # normalized o^T

            def init_v_const(ph):
                for h in range(HPC):
                    col = 64 if h % 2 == 0 else 0
                    nc.vector.tensor_copy(
                        v_sb[ph][:, h, :, col:col + 1],
                        bass.AP(tensor=ones_f.tensor, offset=ones_f.offset,
                                ap=ones_f.ap[:1] + [[0, NJT], [0, 1]]),
                    )
                    if h % 2 == 1:
                        nc.vector.memset(v_sb[ph][:, h, :, 1:64], 0.0)

            def fetch_xs(rep, t):
                tiles = []
                for sp in range(4):
                    xst = stage.tile([128, D], F32R, tag="xs",
                                     name=f"rxs_{rep}_{t}_{sp}")
                    nc.sync.dma_start(
                        out=xst[:],
                        in_=x_d[t * LT + sp * 128: t * LT + (sp + 1) * 128,
                                :].bitcast(F32R))
                    tiles.append(xst)
                return tiles

            def emit_stage1(rep, t, xs):
                """x^T transposes for l-tile t; one closure per k-tile."""
                xT = xtp.tile([128, NKT, LT], F32R, tag="xT", name=f"rxT_{rep}_{t}")
                units = []
                for kt in range(NKT):
                    def u(kt=kt, xT=xT, xs=xs, rep=rep, t=t):
                        tp = ps_tp.tile([128, 2, LT], F32R, tag="s",
                                        name=f"rtp_{rep}_{t}_{kt}")
                        for sp in range(4):
                            nc.tensor.transpose(
                                tp[:, 0, sp * 128:(sp + 1) * 128],
                                xs[sp][:, kt * 128:(kt + 1) * 128], identr[:])
                        nc.vector.tensor_copy(xT[:, kt, :], tp[:, 0, :])
                    units.append(u)
                return xT, units

            def emit_stage2(rep, t, xT, ph):
                """QKV projection closures for l-tile t.  Each c-tile is two
                closures (4 matmuls each) so fillers are fine-grained enough
                to slot one into every attention chunk."""
                l0 = t * LT
                units = []
                for ct in (0, 2, 4, 1, 3, 5):
                    holder = {}
                    def u_a(ct=ct, xT=xT, rep=rep, t=t, holder=holder):
                        pp = ps_pp.tile([128, LT], F32, tag="pp",
                                        name=f"rpp_{rep}_{t}_{ct}")
                        holder["pp"] = pp
                        for kt in range(NKT // 2):
                            nc.tensor.matmul(
                                pp[:], w_sb[:, kt, ct * 128:(ct + 1) * 128],
                                xT[:, kt, :],
                                start=(kt == 0), stop=False)
                    def u_b(ct=ct, xT=xT, rep=rep, t=t, l0=l0, ph=ph,
                            holder=holder):
                        pp = holder["pp"]
                        for kt in range(NKT // 2, NKT):
                            nc.tensor.matmul(
                                pp[:], w_sb[:, kt, ct * 128:(ct + 1) * 128],
                                xT[:, kt, :],
                                start=False, stop=(kt == NKT - 1))
                        if ct < 4:
                            dst = qt_sb[ph] if ct < 2 else kt_sb[ph]
                            nc.vector.tensor_scalar(
                                out=dst[:, ct % 2, l0:l0 + LT], in0=pp[:],
                                scalar1=bq_sb[:, ct:ct + 1], scalar2=None,
                                op0=ADD)
                        else:
                            pv = ct - 4
                            vt_tmp = work.tile([128, LT], F32R, tag="vt_tmp",
                                               name=f"rvt_{rep}_{t}_{pv}")
                            nc.vector.tensor_scalar(
                                out=vt_tmp[:], in0=pp[:],
                                scalar1=bq_sb[:, ct:ct + 1], scalar2=None,
                                op0=ADD)
                            tpv = ps_tp.tile([128, 2, LT], F32R, tag="s",
                                             name=f"rtpv_{rep}_{t}_{pv}")
                            for sp in range(4):
                                nc.tensor.transpose(
                                    tpv[:, 0, sp * 128:(sp + 1) * 128],
                                    vt_tmp[:, sp * 128:(sp + 1) * 128],
                                    identr[:])
                            tpv_v = tpv[:, 0, :].rearrange(
                                "j (sp h d) -> j sp h d", sp=4, h=2)
                            for hh in range(2):
                                c0 = 0 if hh == 0 else 64
                                nc.vector.tensor_copy(
                                    v_sb[ph][:, 2 * pv + hh, 4 * t:4 * (t + 1),
                                             c0:c0 + 64],
                                    tpv_v[:, :, hh, :])
                    units.append(u_a)
                    units.append(u_b)
                return units

            def emit_outproj(rep, t, last):
                """Out-projection closures for i-tile t (8 units)."""
                units = []
                for st in range(4):
                    for mt in range(2):
                        def u(st=st, mt=mt, rep=rep, t=t, last=last):
                            i0 = t * LT + st * 128
                            yp = ps_pp.tile([128, 512], F32, tag="pp",
                                            name=f"ryp_{rep}_{t}_{st}_{mt}")
                            for p in range(2):
                                nc.tensor.matmul(
                                    yp[:], ot_sb[:, p, i0:i0 + 128],
                                    wo_sb[:, p, mt * 512:(mt + 1) * 512],
                                    start=(p == 0), stop=(p == 1))
                            y_sb = work.tile([128, 512], F32, tag="y_sb",
                                             name=f"rysb_{rep}_{t}_{st}_{mt}")
                            if last:
                                nc.scalar.copy(y_sb[:], yp[:])
                            else:
                                nc.vector.tensor_copy(y_sb[:], yp[:])
                            nc.sync.dma_start(
                                out=y_d[i0:i0 + 128, mt * 512:(mt + 1) * 512],
                                in_=y_sb[:])
                        units.append(u)
                return units

            def sc_chunk(rep, t, p, jt, ph):
                """Scores + exp for one key chunk; returns the e tile.  Both
                heads' scores land in one 2-bank PSUM tile so a single ACT
                instruction exponentiates the pair — legal because the skewed
                j-loop gives the exp a full chunk of slack before attn@V
                consumes it."""
                l0 = t * LT
                vis = max(0, jt - 4 * t) * 128
                s2 = ps_s.tile([128, 2, LT], F32, tag="s", bufs=2,
                               name=f"rs_{p}_{rep}_{t}_{jt}")
                for hh in range(2):
                    nc.tensor.matmul(
                        s2[:, hh, vis:LT],
                        kt_sb[ph][hh * 64:(hh + 1) * 64, p,
                                  jt * 128:(jt + 1) * 128],
                        qt_sb[ph][hh * 64:(hh + 1) * 64, p,
                                  l0 + vis:l0 + LT],
                        start=True, stop=True)
                e2 = expp.tile([128, 2, LT], BF16, tag="e_t",
                               name=f"re_{p}_{rep}_{t}_{jt}")
                nc.scalar.activation(
                    e2[:, :, vis:LT], s2[:, :, vis:LT], EXP, scale=SCALE)
                return e2

            def av_chunk(rep, t, p, jt, o_ps, ph, e2):
                njt = 4 * (t + 1)
                vis = max(0, jt - 4 * t) * 128
                for hh in range(2):
                    h = 2 * p + hh
                    if hh == 0:
                        dst = o_ps[hh][0:65, vis:LT]
                        vw = v_sb[ph][:, h, jt, 0:65]
                    else:
                        dst = o_ps[hh][0:128, vis:LT]
                        vw = v_sb[ph][:, h, jt, 0:128]
                    nc.tensor.matmul(
                        dst, vw, e2[:, hh, vis:LT],
                        start=(jt == 0), stop=(jt == njt - 1))

            def attn_normalize_split(rep, t, p, o_ps):
                """Issue the denominator reciprocals now (DVE, overlaps);
                return a closure with the PE broadcast + o^T scaling to be
                drained later in the filler stream."""
                l0 = t * LT
                r2r = small.tile([1, 2, LT], F32R, tag="r2r",
                                 name=f"rr2r_{p}_{rep}_{t}")
                nc.vector.reciprocal(r2r[:, 0, :], o_ps[0][64:65, :])
                nc.vector.reciprocal(r2r[:, 1, :], o_ps[1][0:1, :])

                def nb(rep=rep, t=t, p=p, o_ps=o_ps, r2r=r2r, l0=l0):
                    rb = ps_s.tile([128, 2, LT], F32, tag="s",
                                   name=f"rrb_{p}_{rep}_{t}")
                    nc.tensor.matmul(rb[:, 0, :], sel128r[:], r2r[:, 1, :],
                                     start=True, stop=True)
                    nc.tensor.matmul(rb[0:64, 0, :], ones64r[:], r2r[:, 0, :],
                                     start=True, stop=True)
                    rb_sb = work.tile([128, LT], F32, tag="rb_sb",
                                      name=f"rrbs_{p}_{rep}_{t}")
                    nc.vector.tensor_copy(rb_sb[:], rb[:, 0, :])
                    nc.vector.tensor_mul(
                        ot_sb[0:64, p, l0:l0 + LT],
                        o_ps[0][0:64, :], rb_sb[0:64, :])
                    nc.vector.tensor_mul(
                        ot_sb[64:128, p, l0:l0 + LT],
                        o_ps[1][64:128, :], rb_sb[64:128, :])
                return nb

            def emit_attention(rep, t, fillers, ph, deferred):
                """Attention for i-tile t, software-pipelined two chunks deep:
                scores+exp for chunk jt+2 issue before attn@V of chunk jt
                (4 rotating PSUM score buffers make this legal), so the exp
                and its semaphore hops complete ~2 chunks of PE work early."""
                njt = 4 * (t + 1)
                for p in range(2):
                    o_ps = [ps_o.tile([128, LT], F32, tag="o_ps",
                                      name=f"ro_ps_{p}_{rep}_{t}_{hh}")
                            for hh in range(2)]
                    e_q = [sc_chunk(rep, t, p, 0, ph)]
                    for jt in range(njt):
                        if jt + 1 < njt:
                            e_q.append(sc_chunk(rep, t, p, jt + 1, ph))
                        if fillers:
                            fillers.pop(0)()
                        av_chunk(rep, t, p, jt, o_ps, ph, e_q.pop(0))
                    nb = attn_normalize_split(rep, t, p, o_ps)
                    if p == 0:
                        fillers.insert(0, nb)
                    else:
                        deferred.append(nb)

            xs_next = xs0
            pending = []
            deferred = []
            a_emitted = False
            for rep in range(repeat):
                ph = rep % 2
                for t in range(NLT):
                    if not a_emitted:
                        xT, u1 = emit_stage1(rep, t, xs_next)
                        u2 = emit_stage2(rep, t, xT, ph)
                        u1[0]()
                        if rep == 0:
                            init_v_const(0)
                            if repeat > 1:
                                init_v_const(1)
                        for u in u1[1:] + u2:
                            u()
                    # build the next tile's (or next iteration's) projection
                    # units; they fill PE gaps inside this tile's attention.
                    # Splice two transpose units ahead of the deferred
                    # out-projection so the latter isn't drained right on the
                    # heels of the normalize closure it depends on.
                    if t < NLT - 1:
                        xs_next = fetch_xs(rep, t + 1)
                        xTn, n1 = emit_stage1(rep, t + 1, xs_next)
                        n2 = emit_stage2(rep, t + 1, xTn, ph)
                        pending[1:1] = n1[:2]
                        pending.extend(n1[2:] + n2)
                        a_emitted = True
                    elif rep < repeat - 1:
                        xs_next = fetch_xs(rep + 1, 0)
                        xTn, n1 = emit_stage1(rep + 1, 0, xs_next)
                        n2 = emit_stage2(rep + 1, 0, xTn, 1 - ph)
                        pending[1:1] = n1[:2]
                        pending.extend(n1[2:] + n2)
                        a_emitted = True
                    else:
                        a_emitted = False
                    emit_attention(rep, t, pending, ph, deferred)
                    for u in pending:
                        u()
                    pending.clear()
                    final = (t == NLT - 1 and rep == repeat - 1)
                    up = emit_outproj(rep, t, final)
                    if final:
                        for u in deferred:
                            u()
                        deferred.clear()
                        for u in up:
                            u()
                    else:
                        pending.extend(deferred)
                        deferred.clear()
                        pending.extend(up)
        lp.__exit__(None, None, None)
    nc.compile()
    return nc


_NC_CACHE = {}


def _get_nc():
    if "nc" not in _NC_CACHE:
        _NC_CACHE["nc"] = build_program()
    return _NC_CACHE["nc"]


def make_in_maps(x, W_qkv, b_qkv, W_out):
    x = np.ascontiguousarray(np.asarray(x, dtype=np.float32))
    W_qkv = np.asarray(W_qkv, dtype=np.float32)
    b_qkv = np.asarray(b_qkv, dtype=np.float32)
    W_out = np.asarray(W_out, dtype=np.float32)
    in_maps = []
    for c in range(N_CORES):
        b, g = divmod(c, 4)
        cols = np.concatenate([np.arange(blk * D + g * S, blk * D + (g + 1) * S)
                               for blk in range(3)])
        in_maps.append({
            "x": np.ascontiguousarray(x[b]),
            "w_qkv": np.ascontiguousarray(W_qkv[:, cols]),
            "b_qkv": np.ascontiguousarray(b_qkv[cols]),
            "w_out": np.ascontiguousarray(W_out[g * S:(g + 1) * S, :]),
        })
    return in_maps


def kernel(x, W_qkv, b_qkv, W_out, b_out):
    nc = _get_nc()
    in_maps = make_in_maps(x, W_qkv, b_qkv, W_out)
    res = run_bass_kernel_spmd(nc, in_maps, list(range(N_CORES)))
    b_out = np.asarray(b_out, dtype=np.float32)
    out = np.zeros((B, L, D), dtype=np.float32)
    for c in range(N_CORES):
        out[c // 4] += res.results[c]["y"]
    out += b_out[None, None, :]
    return out
